# revision 30
# baseline (speedup 1.0000x reference)
"""DenseGATv2 Trainium2 kernel (8 NeuronCores, data + sequence parallel).

Problem (hardcoded): B=4, N=1024, D=128, H=8, QKV=16, f32.
  scores[b,i,j,h] = leaky_relu(s_i[b,i,h] + s_j[b,j,h] + edge[b,i,j]*w_e[h])
  alpha = softmax_j(scores);  out = concat_h(alpha_h @ v_h) @ Wo

Sharding: core c -> batch b=c//2, query rows r0=512*(c%2) .. r0+512.
Each core returns its [512, 128] slice; the host concatenates. The device
program is identical on all cores (data-parallel SPMD); only input data
differs (host-sliced).

Per-core dataflow ("transposed scores" layout — S^T tiles are [j=128
partitions, i=512 free] per (head, j-tile)):
  - h^T, h_rows^T via PE transposes (identity matmul); HWDGE loads
  - s_i^T = Wa_i^T @ h_rows^T; s_j = h^T_tiles @ Wa_j; v = h^T_tiles @ Wv
  - edge^T: gpsimd cast-DMA u8->bf16 (exact 0/1) split in two j-halves,
    then DMA xbar transposes — so the main loop starts ~7us in
  - s_i broadcast along the free axis: K=1 ones-matmul into PSUM (exact)
  - pass1 (DVE): S = (edgeT_bf16 * w_e[p,1]) + psum_si   [scalar_tensor_tensor]
  - pass2: L = lrelu(S + s_j[p,1]): mostly ACT Prelu (bias+alpha fused;
      Prelu shares the ACT LUT set with Exp => no table-switch; Lrelu is
      broken on HW — alpha ignored). A fraction runs on GpSimd as
      z = S + s_j; L = max(z, 0.15 z) to offload the ACT engine (GpSimd's
      shared SBUF port only contends with DVE perf-mode ops; all our DVE
      ops are 1x f32 on the dedicated port).
  - pass3 (ACT): E = Exp(L) over exp_group j-tiles at once to amortize the
      352-cycle ACT instruction overhead
  - PV: matmul(psum[17, 512], lhsT=[v_h | 1], rhs=E^T) accumulated over
      j-tiles; the ones-column yields the softmax denominator for free.
      Softmax normalization is deferred to this tiny [16, 512] output.
      Optionally in float32r (full PE rate; ~1.4e-4 elem error).
  - final: matmul(lhsT=outT[:, i-chunk], rhs=Wo) -> [128, 128] out rows.

No max-subtraction in softmax: |scores| <= ~8, exp is safe in f32.

Engine-lane discipline: compute engines cannot move data across partitions
and cannot read PSUM at nonzero partition offsets, so cross-partition
rearrangement (row flattening, broadcast replication, head stacking) goes
through SBUF->SBUF/DRAM DMA, and PSUM reads start at partition 0.
"""

import sys

for _p in ("/opt/trn_rl_repo",):
    if _p not in sys.path:
        sys.path.insert(0, _p)

import numpy as np

import concourse.bacc as bacc
import concourse.tile as tile
import concourse.mybir as mybir
from concourse.bass_utils import run_bass_kernel_spmd

F32 = mybir.dt.float32
F32R = mybir.dt.float32r
BF16 = mybir.dt.bfloat16
U8 = mybir.dt.uint8

B, N, D, H, QKV = 4, 1024, 128, 8, 16
NEG_SLOPE = 0.15
N_CORES = 8
ROWS = 512               # query rows per core
P = 128
N_JT = N // P            # 8 key tiles
N_IC = ROWS // P         # 4 query-row chunks
ALU = mybir.AluOpType
ACTF = mybir.ActivationFunctionType

# tuning knobs (read at build time)
CFG = {
    "exp_group": 4,      # j-tiles per Exp instruction
    "work_bufs": 4,
    "quad_bufs": 2,
    "epi_bufs": 2,
    "bc_bufs": 2,
    "po_bufs": 3,
    "gp_lrelu_mod": 0,   # every k-th (h,jt) tile's lrelu runs on GpSimd; 0=off
    "dve_lrelu_mod": 0,  # every k-th tile's lrelu on DVE (2-op max form); 0=off
    "pv_f32r": False,    # PV + final matmuls in float32r (4x PE rate)
}

_cache = {}


def _build_program(local_only: int):
    nc = bacc.Bacc("TRN2", target_bir_lowering=False, debug=False)

    h_d = nc.dram_tensor("h_b", [N, D], F32, kind="ExternalInput")
    hr_d = nc.dram_tensor("h_rows", [ROWS, D], F32, kind="ExternalInput")
    sc_d = nc.dram_tensor("sc_rows", [ROWS, N], U8, kind="ExternalInput")
    wa_d = nc.dram_tensor("Wa", [2 * D + 1, H], F32, kind="ExternalInput")
    wv_d = nc.dram_tensor("Wv", [D, H * QKV], F32, kind="ExternalInput")
    wo_d = nc.dram_tensor("Wo", [H * QKV, D], F32, kind="ExternalInput")
    eye_d = nc.dram_tensor("eye", [P, P], F32, kind="ExternalInput")
    ones_d = nc.dram_tensor("ones", [1, ROWS], F32, kind="ExternalInput")
    out_d = nc.dram_tensor("out_rows", [ROWS, D], F32, kind="ExternalOutput")

    PVDT = F32R if CFG["pv_f32r"] else F32
    EG = CFG["exp_group"]
    GPM = CFG["gp_lrelu_mod"]
    DVM = CFG["dve_lrelu_mod"]

    with tile.TileContext(nc) as tc:
        with (
            tc.tile_pool(name="consts", bufs=1) as consts,
            tc.tile_pool(name="big", bufs=1) as big,
            tc.tile_pool(name="work", bufs=CFG["work_bufs"]) as work,
            tc.tile_pool(name="quad", bufs=CFG["quad_bufs"]) as quad,
            tc.tile_pool(name="epi", bufs=CFG["epi_bufs"]) as epi,
            tc.tile_pool(name="dsc", bufs=2, space="DRAM") as dsc,
            tc.tile_pool(name="ps_a", bufs=3, space="PSUM") as ps_a,
            tc.tile_pool(name="ps_bc", bufs=CFG["bc_bufs"], space="PSUM") as ps_bc,
            tc.tile_pool(name="ps_out", bufs=CFG["po_bufs"], space="PSUM") as ps_out,
        ):
            # ---- setup. Emission order = scheduling priority: the bc/si
            # chain first (gates the first STT), then per-jt rounds so
            # j-tile 0's edgeT / v / s_j are ready earliest.
            # constants / weights (SWDGE: tiny)
            eye_sb = consts.tile([P, P], F32, tag="eye")
            nc.gpsimd.dma_start(out=eye_sb, in_=eye_d.ap())
            eye_bf = consts.tile([P, P], BF16, tag="eye_bf")
            nc.gpsimd.dma_start(out=eye_bf, in_=eye_d.ap())
            ones_sb = consts.tile([1, ROWS], F32, tag="ones")
            nc.gpsimd.dma_start(out=ones_sb, in_=ones_d.ap())
            wai_sb = consts.tile([P, H], F32, tag="wai")
            nc.gpsimd.dma_start(out=wai_sb, in_=wa_d.ap()[0:D, :])
            we_row = consts.tile([1, H], F32, tag="we_row")
            nc.gpsimd.dma_start(out=we_row, in_=wa_d.ap()[2 * D:2 * D + 1, :])
            # Wa_j and Wv concatenated -> one matmul per j-tile for s_j|v
            wjv_sb = consts.tile([P, H + H * QKV], F32, tag="wjv")
            nc.gpsimd.dma_start(out=wjv_sb[:, 0:H], in_=wa_d.ap()[D:2 * D, :])
            nc.gpsimd.dma_start(out=wjv_sb[:, H:], in_=wv_d.ap())
            wo_sb = consts.tile([P, D], PVDT, tag="wo")
            nc.gpsimd.dma_start(out=wo_sb.bitcast(F32), in_=wo_d.ap())

            we_bc = consts.tile([P, H], F32, tag="we_bc")
            nc.gpsimd.partition_broadcast(we_bc[:], we_row[0:1, :])

            # hr chain -> s_i^T flattened (gates bc matmuls)
            hr_nat = big.tile([P, N_IC, D], F32, tag="hr_nat")
            nc.scalar.dma_start(
                out=hr_nat, in_=hr_d.ap().rearrange("(t p) d -> p t d", p=P)
            )
            hrT = big.tile([P, ROWS], F32, tag="hrT")  # [D, ROWS]
            for t in range(N_IC):
                pst = ps_a.tile([P, P], F32, tag="pss", name=f"psthr_{t}")
                nc.tensor.transpose(pst[:], hr_nat[:, t, :], eye_sb[:])
                nc.vector.tensor_copy(out=hrT[:, t * P:(t + 1) * P], in_=pst)
            ps_si = ps_a.tile([H, ROWS], F32, tag="pss", name="ps_si")
            nc.tensor.matmul(ps_si, wai_sb[:, :], hrT[:, :], start=True, stop=True)
            siT = consts.tile([H, ROWS], F32, tag="siT")
            nc.vector.tensor_copy(out=siT, in_=ps_si)
            siT_flat = consts.tile([1, H * ROWS], F32, tag="siT_flat")
            nc.sync.dma_start(out=siT_flat, in_=siT[:, :])

            # edge loads + converts (engines split), h loads
            sc_u8 = []
            sc_bf = []
            for it in range(N_IC):
                u8t = big.tile([P, N], U8, tag=f"sc_u8_{it}", name=f"sc_u8_{it}")
                (nc.sync if it % 2 == 0 else nc.scalar).dma_start(
                    out=u8t, in_=sc_d.ap()[it * P:(it + 1) * P, :])
                sc_u8.append(u8t)
                bft = big.tile([P, N], BF16, tag=f"sc_bf_{it}", name=f"sc_bf_{it}")
                if it % 2 == 0:
                    nc.vector.tensor_copy(out=bft, in_=u8t)
                else:
                    nc.scalar.copy(out=bft, in_=u8t)
                sc_bf.append(bft)
            h_nat = big.tile([P, N_JT, D], F32, tag="h_nat")
            nc.sync.dma_start(
                out=h_nat, in_=h_d.ap().rearrange("(t p) d -> p t d", p=P)
            )

            # per-jt rounds: h^T block, edge^T block, s_j|v matmul
            hT = big.tile([P, N], F32, tag="hT")  # [D, N]
            edgeT = []
            sj_all = consts.tile([P, N_JT, H], F32, tag="sj_all")
            v_ones = big.tile([P, N_JT, H, QKV + 1], PVDT, tag="v_ones")
            for t in range(N_JT):
                pst = ps_a.tile([P, P], F32, tag="pss", name=f"psth_{t}")
                nc.tensor.transpose(pst[:], h_nat[:, t, :], eye_sb[:])
                nc.vector.tensor_copy(out=hT[:, t * P:(t + 1) * P], in_=pst)

                et = big.tile([P, ROWS], BF16, tag=f"edgeT_{t}", name=f"edgeT_{t}")
                pse = ps_a.tile([P, ROWS], BF16, tag="pss", name=f"pse_{t}")
                for it in range(N_IC):
                    nc.tensor.matmul(
                        pse[:, it * P:(it + 1) * P],
                        sc_bf[it][:, t * P:(t + 1) * P], eye_bf[:],
                        is_transpose=True, start=True, stop=True,
                    )
                if t % 2 == 0:
                    nc.vector.tensor_copy(out=et, in_=pse)
                else:
                    nc.scalar.copy(out=et, in_=pse)
                edgeT.append(et)

                ps_jv = ps_a.tile([P, H + H * QKV], F32, tag="pss",
                                  name=f"ps_jv_{t}")
                nc.tensor.matmul(
                    ps_jv, hT[:, t * P:(t + 1) * P], wjv_sb[:, :],
                    start=True, stop=True,
                )
                nc.scalar.copy(out=sj_all[:, t, :], in_=ps_jv[:, 0:H])
                nc.scalar.copy(
                    out=v_ones[:, t, :, 0:QKV],
                    in_=ps_jv[:, H:].rearrange("p (h q) -> p h q", h=H),
                )
                nc.gpsimd.memset(v_ones[:, t, :, QKV:QKV + 1], 1.0)

            sj15 = None
            if DVM:
                sj15 = consts.tile([P, N_JT, H], F32, tag="sj15")
                nc.vector.tensor_scalar(
                    sj15.rearrange("p a b -> p (a b)"),
                    sj_all.rearrange("p a b -> p (a b)"),
                    NEG_SLOPE, None, op0=ALU.mult,
                )

            # ---- main loop: heads x j-tile groups ------------------------
            outT = big.tile([P, ROWS], PVDT, tag="outT")  # [H*QKV, ROWS]
            tile_no = 0

            def emit_head_epilogue(h, po_h):
                # normalize rows 0..15 of po_h by 1/row16 -> outT.
                pt = epi.tile([QKV + 1, ROWS], F32, tag="pt")
                nc.vector.tensor_copy(out=pt, in_=po_h)
                rsc = dsc.tile([1, ROWS], F32, tag="rsc")
                nc.sync.dma_start(out=rsc, in_=pt[QKV:QKV + 1, :])
                den16 = epi.tile([QKV, ROWS], F32, tag="den16")
                nc.sync.dma_start(
                    out=den16, in_=rsc[0:1, :].to_broadcast([QKV, ROWS])
                )
                rec16 = epi.tile([QKV, ROWS], F32, tag="rec16")
                nc.vector.reciprocal(out=rec16, in_=den16)
                on_h = epi.tile([QKV, ROWS], PVDT, tag="on_h")
                nc.gpsimd.tensor_tensor(
                    out=on_h, in0=pt[0:QKV, :], in1=rec16, op=ALU.mult
                )
                nc.sync.dma_start(
                    out=outT[h * QKV:(h + 1) * QKV, :], in_=on_h
                )

            pending_epi = []  # epilogue deferred one head for pipelining
            bc_tiles = {}

            def ensure_bc(hh):
                if hh < H and hh not in bc_tiles:
                    t = ps_bc.tile([P, ROWS], F32, tag="bc", name=f"bc_{hh}")
                    nc.tensor.matmul(
                        t, ones_sb[0:1, 0:P],
                        siT_flat[0:1, hh * ROWS:(hh + 1) * ROWS],
                        start=True, stop=True,
                    )
                    bc_tiles[hh] = t

            for h in range(H):
                ensure_bc(h)
                bc_h = bc_tiles.pop(h)
                po_h = ps_out.tile([QKV + 1, ROWS], F32, tag="po",
                                   name=f"po_{h}")
                for g in range(N_JT // EG):
                    lq = quad.tile([P, EG * ROWS], F32, tag="lq")
                    for q in range(EG):
                        jt = g * EG + q
                        s_t = work.tile([P, ROWS], F32, tag="s")
                        nc.vector.scalar_tensor_tensor(
                            out=s_t, in0=edgeT[jt][:, :],
                            scalar=we_bc[:, h:h + 1], in1=bc_h,
                            op0=ALU.mult, op1=ALU.add,
                        )
                        if g == 1 and q == 2 and pending_epi:
                            emit_head_epilogue(*pending_epi.pop())
                        ldst = lq[:, q * ROWS:(q + 1) * ROWS]
                        use_gp = GPM and (tile_no % GPM == 0)
                        use_dve = (not use_gp) and DVM and (tile_no % DVM == 0)
                        tile_no += 1
                        if use_dve:
                            t15 = work.tile([P, ROWS], F32, tag="t15")
                            nc.vector.tensor_scalar(
                                t15, s_t, NEG_SLOPE,
                                sj15[:, jt, h:h + 1],
                                op0=ALU.mult, op1=ALU.add,
                            )
                            nc.vector.scalar_tensor_tensor(
                                out=ldst, in0=s_t,
                                scalar=sj_all[:, jt, h:h + 1], in1=t15,
                                op0=ALU.add, op1=ALU.max,
                            )
                        elif use_gp:
                            z_t = work.tile([P, ROWS], F32, tag="z")
                            nc.gpsimd.tensor_scalar(
                                z_t, s_t, sj_all[:, jt, h:h + 1], None,
                                op0=ALU.add,
                            )
                            nc.gpsimd.scalar_tensor_tensor(
                                out=ldst, in0=z_t, scalar=NEG_SLOPE, in1=z_t,
                                op0=ALU.mult, op1=ALU.max,
                            )
                        else:
                            nc.scalar.activation(
                                out=ldst, in_=s_t, func=ACTF.Prelu,
                                bias=sj_all[:, jt, h:h + 1],
                                scale=1.0, alpha=NEG_SLOPE,
                            )
                    eq = quad.tile([P, EG * ROWS], PVDT, tag="eq")
                    nc.scalar.activation(out=eq, in_=lq, func=ACTF.Exp)
                    for q in range(EG):
                        jt = g * EG + q
                        if local_only:
                            nc.vector.tensor_tensor(
                                out=eq[:, q * ROWS:(q + 1) * ROWS],
                                in0=eq[:, q * ROWS:(q + 1) * ROWS],
                                in1=edgeT[jt][:, :], op=ALU.mult,
                            )
                        nc.tensor.matmul(
                            po_h, v_ones[:, jt, h, :],
                            eq[:, q * ROWS:(q + 1) * ROWS],
                            start=(jt == 0), stop=(jt == N_JT - 1),
                        )
                    if g == 0:
                        ensure_bc(h + 1)
                pending_epi.append((h, po_h))
            while pending_epi:
                emit_head_epilogue(*pending_epi.pop())

            # ---- final projection ----------------------------------------
            for ic in range(N_IC):
                psf = ps_a.tile([P, D], F32, tag="pss", name=f"psf_{ic}")
                nc.tensor.matmul(
                    psf, outT[:, ic * P:(ic + 1) * P], wo_sb[:, :],
                    start=True, stop=True,
                )
                fin = work.tile([P, D], F32, tag="fin")
                nc.vector.tensor_copy(out=fin, in_=psf)
                nc.sync.dma_start(out=out_d.ap()[ic * P:(ic + 1) * P, :], in_=fin)

    nc.compile()
    return nc


def _make_in_maps(inputs):
    h = np.ascontiguousarray(np.asarray(inputs["h"], dtype=np.float32))
    sc = np.asarray(inputs["same_cluster"])
    if sc.dtype != np.uint8:
        sc = sc.astype(np.uint8)
    Wa = np.ascontiguousarray(np.asarray(inputs["Wa"], dtype=np.float32))
    Wv = np.ascontiguousarray(np.asarray(inputs["Wv"], dtype=np.float32))
    Wo = np.ascontiguousarray(np.asarray(inputs["Wo"], dtype=np.float32))
    eye = np.eye(P, dtype=np.float32)
    ones = np.ones((1, ROWS), dtype=np.float32)

    in_maps = []
    for c in range(N_CORES):
        b = c // 2
        r0 = (c % 2) * ROWS
        in_maps.append({
            "h_b": h[b],
            "h_rows": np.ascontiguousarray(h[b, r0:r0 + ROWS, :]),
            "sc_rows": np.ascontiguousarray(sc[b, r0:r0 + ROWS, :]),
            "Wa": Wa, "Wv": Wv, "Wo": Wo, "eye": eye, "ones": ones,
        })
    return in_maps


def _build_runner(nc):
    """Persistent jitted shard_map runner (avoids per-call retracing)."""
    import jax
    from jax.sharding import Mesh, PartitionSpec
    from jax.experimental.shard_map import shard_map
    from concourse.bass2jax import (
        _bass_exec_p, install_neuronx_cc_hook, partition_id_tensor,
    )

    install_neuronx_cc_hook()
    partition_name = nc.partition_id_tensor.name if nc.partition_id_tensor else None
    in_names, out_names, out_avals, zero_shapes = [], [], [], []
    for alloc in nc.m.functions[0].allocations:
        if not isinstance(alloc, mybir.MemoryLocationSet):
            continue
        name = alloc.memorylocations[0].name
        if alloc.kind == "ExternalInput":
            if name != partition_name:
                in_names.append(name)
        elif alloc.kind == "ExternalOutput":
            out_names.append(name)
            shape = tuple(alloc.tensor_shape)
            dtype = mybir.dt.np(alloc.dtype)
            out_avals.append(jax.core.ShapedArray(shape, dtype))
            zero_shapes.append((shape, dtype))
    n_params = len(in_names)
    all_in_names = list(in_names) + list(out_names)
    if partition_name is not None:
        all_in_names.append(partition_name)

    def _body(*args):
        operands = list(args)
        if partition_name is not None:
            operands.append(partition_id_tensor())
        outs = _bass_exec_p.bind(
            *operands,
            out_avals=tuple(out_avals),
            in_names=tuple(all_in_names),
            out_names=tuple(out_names),
            lowering_input_output_aliases=(),
            sim_require_finite=True,
            sim_require_nnan=True,
            nc=nc,
        )
        return tuple(outs)

    devices = jax.devices()[:N_CORES]
    mesh = Mesh(np.asarray(devices), ("core",))
    in_specs = (PartitionSpec("core"),) * (n_params + len(out_names))
    out_specs = (PartitionSpec("core"),) * len(out_names)
    fn = jax.jit(
        shard_map(_body, mesh=mesh, in_specs=in_specs, out_specs=out_specs,
                  check_rep=False),
        donate_argnums=tuple(range(n_params, n_params + len(out_names))),
        keep_unused=True,
    )
    return fn, in_names, out_names, zero_shapes


def kernel(h, same_cluster, Wa, Wv, Wo, local_only):
    local_only = int(local_only)
    key = ("prog", local_only)
    if key not in _cache:
        _cache[key] = _build_program(local_only)
    nc = _cache[key]

    in_maps = _make_in_maps({
        "h": h, "same_cluster": same_cluster, "Wa": Wa, "Wv": Wv, "Wo": Wo,
    })

    try:
        rkey = ("runner", local_only)
        if rkey not in _cache:
            _cache[rkey] = _build_runner(nc)
        fn, in_names, out_names, zero_shapes = _cache[rkey]
        concat_in = [
            np.concatenate([np.asarray(in_maps[c][nm]) for c in range(N_CORES)],
                           axis=0)
            for nm in in_names
        ]
        concat_zeros = [
            np.zeros((N_CORES * s[0], *s[1:]), dt) for s, dt in zero_shapes
        ]
        out_arrs = fn(*concat_in, *concat_zeros)
        res_per_core = np.asarray(out_arrs[out_names.index("out_rows")]).reshape(
            N_CORES, ROWS, D
        )
    except Exception:
        res = run_bass_kernel_spmd(nc, in_maps, list(range(N_CORES)))
        res_per_core = np.stack(
            [res.results[c]["out_rows"] for c in range(N_CORES)]
        )

    out = np.empty((B, N, D), dtype=np.float32)
    for c in range(N_CORES):
        b = c // 2
        r0 = (c % 2) * ROWS
        out[b, r0:r0 + ROWS, :] = res_per_core[c]
    return out


if __name__ == "__main__":
    rng = np.random.default_rng(0)
    h = rng.standard_normal((B, N, D), dtype=np.float32)
    sc = rng.integers(0, 2, (B, N, N)).astype(bool)
    Wa = rng.standard_normal((2 * D + 1, H), dtype=np.float32) / np.sqrt(2 * D + 1)
    Wv = rng.standard_normal((D, H * QKV), dtype=np.float32) / np.sqrt(D)
    Wo = rng.standard_normal((H * QKV, D), dtype=np.float32) / np.sqrt(H * QKV)

    out = kernel(h=h, same_cluster=sc, Wa=Wa, Wv=Wv, Wo=Wo, local_only=0)

    Wa_i, Wa_j, w_e = Wa[:D], Wa[D:2 * D], Wa[2 * D]
    s_i = h @ Wa_i
    s_j = h @ Wa_j
    scores = (s_i[:, :, None, :] + s_j[:, None, :, :]
              + sc.astype(np.float32)[..., None] * w_e)
    scores = np.where(scores > 0, scores, NEG_SLOPE * scores)
    scores = np.moveaxis(scores, -1, 1)
    scores = scores - scores.max(axis=-1, keepdims=True)
    e = np.exp(scores)
    alpha = e / e.sum(axis=-1, keepdims=True)
    v = (h @ Wv).reshape(B, N, H, QKV).transpose(0, 2, 1, 3)
    o = np.einsum('bhij,bhjd->bhid', alpha, v)
    o = o.transpose(0, 2, 1, 3).reshape(B, N, H * QKV)
    expected = o @ Wo

    err = np.abs(out - expected)
    rel = np.linalg.norm(out - expected) / np.linalg.norm(expected)
    print(f"rel_err(norm)={rel:.3e} max_abs={err.max():.3e}")


# revision 32
# speedup vs baseline: 1.0129x; 1.0129x over previous
"""DenseGATv2 Trainium2 kernel (8 NeuronCores, data + sequence parallel).

Problem (hardcoded): B=4, N=1024, D=128, H=8, QKV=16, f32.
  scores[b,i,j,h] = leaky_relu(s_i[b,i,h] + s_j[b,j,h] + edge[b,i,j]*w_e[h])
  alpha = softmax_j(scores);  out = concat_h(alpha_h @ v_h) @ Wo

Steady state is ACT-bound (~4.3us per half-head cycle: 4 Prelus + 1 big
Exp); cost-model makespan ~113us/core.

Sharding: core c -> batch b=c//2, query rows r0=512*(c%2) .. r0+512.
Each core returns its [512, 128] slice; the host concatenates. The device
program is identical on all cores (data-parallel SPMD); only input data
differs (host-sliced).

Per-core dataflow ("transposed scores" layout — S^T tiles are [j=128
partitions, i=512 free] per (head, j-tile)):
  - h^T, h_rows^T via PE transposes (identity matmul); HWDGE loads
  - s_i^T = Wa_i^T @ h_rows^T; s_j|v = h^T_tiles @ [Wa_j | Wv] (one fused
    matmul per j-tile)
  - edge^T: HWDGE u8 loads, u8->bf16 converts split across DVE/ACT (both
    idle during setup; 0/1 exact in bf16), then PE transposes batched four
    [128,128] blocks into one [128,512] PSUM tile with a single copy out
  - s_i broadcast along the free axis: K=1 ones-matmul into PSUM (exact)
  - pass1 (DVE): S = (edgeT_bf16 * w_e[p,1]) + psum_si   [scalar_tensor_tensor]
  - pass2 (ACT): L = Prelu(S + s_j[p,1], alpha=0.15) — bias+alpha fused in
      one op; Prelu shares the ACT LUT set with Exp so there is no
      table-switch cost (Lrelu is broken on HW — its alpha is ignored).
      Optional knobs can route some tiles' lrelu to GpSimd/DVE, but the
      schedule is latency-bound there and they don't help (default off).
  - pass3 (ACT): E = Exp(L) over exp_group j-tiles at once to amortize the
      352-cycle ACT instruction overhead
  - PV: matmul(psum[17, 512], lhsT=[v_h | 1], rhs=E^T) accumulated over
      j-tiles; the ones-column yields the softmax denominator for free.
      Softmax normalization is deferred to this tiny [16, 512] output.
      Optionally in float32r (full PE rate; ~1.4e-4 elem error).
  - final: matmul(lhsT=outT[:, i-chunk], rhs=Wo) -> [128, 128] out rows.

No max-subtraction in softmax: |scores| <= ~8, exp is safe in f32.

Engine-lane discipline: compute engines cannot move data across partitions
and cannot read PSUM at nonzero partition offsets, so cross-partition
rearrangement (row flattening, broadcast replication, head stacking) goes
through SBUF->SBUF/DRAM DMA, and PSUM reads start at partition 0.
"""

import sys

for _p in ("/opt/trn_rl_repo",):
    if _p not in sys.path:
        sys.path.insert(0, _p)

import numpy as np

import concourse.bacc as bacc
import concourse.tile as tile
import concourse.mybir as mybir
from concourse.bass_utils import run_bass_kernel_spmd

F32 = mybir.dt.float32
F32R = mybir.dt.float32r
BF16 = mybir.dt.bfloat16
U8 = mybir.dt.uint8

B, N, D, H, QKV = 4, 1024, 128, 8, 16
NEG_SLOPE = 0.15
N_CORES = 8
ROWS = 512               # query rows per core
P = 128
N_JT = N // P            # 8 key tiles
N_IC = ROWS // P         # 4 query-row chunks
ALU = mybir.AluOpType
ACTF = mybir.ActivationFunctionType

# tuning knobs (read at build time)
CFG = {
    "exp_group": 4,      # j-tiles per Exp instruction
    "work_bufs": 4,
    "quad_bufs": 2,
    "epi_bufs": 2,
    "bc_bufs": 3,
    "po_bufs": 2,
    "gp_lrelu_mod": 0,   # every k-th (h,jt) tile's lrelu runs on GpSimd; 0=off
    "dve_lrelu_mod": 0,  # every k-th tile's lrelu on DVE (2-op max form); 0=off
    "pv_f32r": False,    # PV + final matmuls in float32r (4x PE rate)
}

_cache = {}


def _build_program(local_only: int):
    nc = bacc.Bacc("TRN2", target_bir_lowering=False, debug=False)

    h_d = nc.dram_tensor("h_b", [N, D], F32, kind="ExternalInput")
    hr_d = nc.dram_tensor("h_rows", [ROWS, D], F32, kind="ExternalInput")
    sc_d = nc.dram_tensor("sc_rows", [ROWS, N], U8, kind="ExternalInput")
    wa_d = nc.dram_tensor("Wa", [2 * D + 1, H], F32, kind="ExternalInput")
    wv_d = nc.dram_tensor("Wv", [D, H * QKV], F32, kind="ExternalInput")
    wo_d = nc.dram_tensor("Wo", [H * QKV, D], F32, kind="ExternalInput")
    eye_d = nc.dram_tensor("eye", [P, P], F32, kind="ExternalInput")
    ones_d = nc.dram_tensor("ones", [1, ROWS], F32, kind="ExternalInput")
    out_d = nc.dram_tensor("out_rows", [ROWS, D], F32, kind="ExternalOutput")

    PVDT = F32R if CFG["pv_f32r"] else F32
    EG = CFG["exp_group"]
    GPM = CFG["gp_lrelu_mod"]
    DVM = CFG["dve_lrelu_mod"]

    with tile.TileContext(nc) as tc:
        with (
            tc.tile_pool(name="consts", bufs=1) as consts,
            tc.tile_pool(name="big", bufs=1) as big,
            tc.tile_pool(name="work", bufs=CFG["work_bufs"]) as work,
            tc.tile_pool(name="quad", bufs=CFG["quad_bufs"]) as quad,
            tc.tile_pool(name="epi", bufs=CFG["epi_bufs"]) as epi,
            tc.tile_pool(name="dsc", bufs=2, space="DRAM") as dsc,
            tc.tile_pool(name="ps_a", bufs=3, space="PSUM") as ps_a,
            tc.tile_pool(name="ps_bc", bufs=CFG["bc_bufs"], space="PSUM") as ps_bc,
            tc.tile_pool(name="ps_out", bufs=CFG["po_bufs"], space="PSUM") as ps_out,
        ):
            # ---- setup. Emission order = scheduling priority: the bc/si
            # chain first (gates the first STT), then per-jt rounds so
            # j-tile 0's edgeT / v / s_j are ready earliest.
            # constants / weights (SWDGE: tiny)
            eye_sb = consts.tile([P, P], F32, tag="eye")
            nc.gpsimd.dma_start(out=eye_sb, in_=eye_d.ap())
            eye_bf = consts.tile([P, P], BF16, tag="eye_bf")
            nc.gpsimd.dma_start(out=eye_bf, in_=eye_d.ap())
            ones_sb = consts.tile([1, ROWS], F32, tag="ones")
            nc.gpsimd.dma_start(out=ones_sb, in_=ones_d.ap())
            wai_sb = consts.tile([P, H], F32, tag="wai")
            nc.gpsimd.dma_start(out=wai_sb, in_=wa_d.ap()[0:D, :])
            we_row = consts.tile([1, H], F32, tag="we_row")
            nc.gpsimd.dma_start(out=we_row, in_=wa_d.ap()[2 * D:2 * D + 1, :])
            # Wa_j and Wv concatenated -> one matmul per j-tile for s_j|v
            wjv_sb = consts.tile([P, H + H * QKV], F32, tag="wjv")
            nc.gpsimd.dma_start(out=wjv_sb[:, 0:H], in_=wa_d.ap()[D:2 * D, :])
            nc.gpsimd.dma_start(out=wjv_sb[:, H:], in_=wv_d.ap())
            wo_sb = consts.tile([P, D], PVDT, tag="wo")
            nc.gpsimd.dma_start(out=wo_sb.bitcast(F32), in_=wo_d.ap())

            we_bc = consts.tile([P, H], F32, tag="we_bc")
            nc.gpsimd.partition_broadcast(we_bc[:], we_row[0:1, :])

            # hr chain -> s_i^T flattened (gates bc matmuls)
            hr_nat = big.tile([P, N_IC, D], F32, tag="hr_nat")
            nc.scalar.dma_start(
                out=hr_nat, in_=hr_d.ap().rearrange("(t p) d -> p t d", p=P)
            )
            hrT = big.tile([P, ROWS], F32, tag="hrT")  # [D, ROWS]
            for t in range(N_IC):
                pst = ps_a.tile([P, P], F32, tag="pss", name=f"psthr_{t}")
                nc.tensor.transpose(pst[:], hr_nat[:, t, :], eye_sb[:])
                nc.vector.tensor_copy(out=hrT[:, t * P:(t + 1) * P], in_=pst)
            ps_si = ps_a.tile([H, ROWS], F32, tag="pss", name="ps_si")
            nc.tensor.matmul(ps_si, wai_sb[:, :], hrT[:, :], start=True, stop=True)
            siT = consts.tile([H, ROWS], F32, tag="siT")
            nc.vector.tensor_copy(out=siT, in_=ps_si)
            siT_flat = consts.tile([1, H * ROWS], F32, tag="siT_flat")
            nc.sync.dma_start(out=siT_flat, in_=siT[:, :])

            # edge loads + converts (engines split), h loads
            sc_u8 = []
            sc_bf = []
            for it in range(N_IC):
                u8t = big.tile([P, N], U8, tag=f"sc_u8_{it}", name=f"sc_u8_{it}")
                (nc.sync if it % 2 == 0 else nc.scalar).dma_start(
                    out=u8t, in_=sc_d.ap()[it * P:(it + 1) * P, :])
                sc_u8.append(u8t)
                bft = big.tile([P, N], BF16, tag=f"sc_bf_{it}", name=f"sc_bf_{it}")
                if it % 2 == 0:
                    nc.vector.tensor_copy(out=bft, in_=u8t)
                else:
                    nc.scalar.copy(out=bft, in_=u8t)
                sc_bf.append(bft)
            h_nat = big.tile([P, N_JT, D], F32, tag="h_nat")
            nc.sync.dma_start(
                out=h_nat, in_=h_d.ap().rearrange("(t p) d -> p t d", p=P)
            )

            # per-jt rounds: h^T block, edge^T block, s_j|v matmul
            hT = big.tile([P, N], F32, tag="hT")  # [D, N]
            edgeT = []
            sj_all = consts.tile([P, N_JT, H], F32, tag="sj_all")
            v_ones = big.tile([P, N_JT, H, QKV + 1], PVDT, tag="v_ones")
            for t in range(N_JT):
                pst = ps_a.tile([P, P], F32, tag="pss", name=f"psth_{t}")
                nc.tensor.transpose(pst[:], h_nat[:, t, :], eye_sb[:])
                nc.vector.tensor_copy(out=hT[:, t * P:(t + 1) * P], in_=pst)

                et = big.tile([P, ROWS], BF16, tag=f"edgeT_{t}", name=f"edgeT_{t}")
                pse = ps_a.tile([P, ROWS], BF16, tag="pss", name=f"pse_{t}")
                for it in range(N_IC):
                    nc.tensor.matmul(
                        pse[:, it * P:(it + 1) * P],
                        sc_bf[it][:, t * P:(t + 1) * P], eye_bf[:],
                        is_transpose=True, start=True, stop=True,
                    )
                if t % 2 == 0:
                    nc.vector.tensor_copy(out=et, in_=pse)
                else:
                    nc.scalar.copy(out=et, in_=pse)
                edgeT.append(et)

                ps_jv = ps_a.tile([P, H + H * QKV], F32, tag="pss",
                                  name=f"ps_jv_{t}")
                nc.tensor.matmul(
                    ps_jv, hT[:, t * P:(t + 1) * P], wjv_sb[:, :],
                    start=True, stop=True,
                )
                nc.scalar.copy(out=sj_all[:, t, :], in_=ps_jv[:, 0:H])
                nc.scalar.copy(
                    out=v_ones[:, t, :, 0:QKV],
                    in_=ps_jv[:, H:].rearrange("p (h q) -> p h q", h=H),
                )
                nc.gpsimd.memset(v_ones[:, t, :, QKV:QKV + 1], 1.0)

            sj15 = None
            if DVM:
                sj15 = consts.tile([P, N_JT, H], F32, tag="sj15")
                nc.vector.tensor_scalar(
                    sj15.rearrange("p a b -> p (a b)"),
                    sj_all.rearrange("p a b -> p (a b)"),
                    NEG_SLOPE, None, op0=ALU.mult,
                )

            # ---- main loop: heads x j-tile groups ------------------------
            outT = big.tile([P, ROWS], PVDT, tag="outT")  # [H*QKV, ROWS]
            tile_no = 0

            def emit_head_epilogue(h, po_h):
                # normalize rows 0..15 of po_h by 1/row16 -> outT.
                pt = epi.tile([QKV + 1, ROWS], F32, tag="pt")
                nc.vector.tensor_copy(out=pt, in_=po_h)
                rsc = dsc.tile([1, ROWS], F32, tag="rsc")
                nc.sync.dma_start(out=rsc, in_=pt[QKV:QKV + 1, :])
                den16 = epi.tile([QKV, ROWS], F32, tag="den16")
                nc.sync.dma_start(
                    out=den16, in_=rsc[0:1, :].to_broadcast([QKV, ROWS])
                )
                rec16 = epi.tile([QKV, ROWS], F32, tag="rec16")
                nc.vector.reciprocal(out=rec16, in_=den16)
                on_h = epi.tile([QKV, ROWS], PVDT, tag="on_h")
                nc.gpsimd.tensor_tensor(
                    out=on_h, in0=pt[0:QKV, :], in1=rec16, op=ALU.mult
                )
                nc.sync.dma_start(
                    out=outT[h * QKV:(h + 1) * QKV, :], in_=on_h
                )

            pending_epi = []  # epilogue deferred one head for pipelining
            bc_tiles = {}

            def ensure_bc(hh):
                if hh < H and hh not in bc_tiles:
                    t = ps_bc.tile([P, ROWS], F32, tag="bc", name=f"bc_{hh}")
                    nc.tensor.matmul(
                        t, ones_sb[0:1, 0:P],
                        siT_flat[0:1, hh * ROWS:(hh + 1) * ROWS],
                        start=True, stop=True,
                    )
                    bc_tiles[hh] = t

            for h in range(H):
                ensure_bc(h)
                bc_h = bc_tiles.pop(h)
                po_h = ps_out.tile([QKV + 1, ROWS], F32, tag="po",
                                   name=f"po_{h}")
                for g in range(N_JT // EG):
                    lq = quad.tile([P, EG * ROWS], F32, tag="lq")
                    for q in range(EG):
                        jt = g * EG + q
                        s_t = work.tile([P, ROWS], F32, tag="s")
                        nc.vector.scalar_tensor_tensor(
                            out=s_t, in0=edgeT[jt][:, :],
                            scalar=we_bc[:, h:h + 1], in1=bc_h,
                            op0=ALU.mult, op1=ALU.add,
                        )
                        if g == 1 and q == 2 and pending_epi:
                            emit_head_epilogue(*pending_epi.pop())
                        ldst = lq[:, q * ROWS:(q + 1) * ROWS]
                        use_gp = GPM and (tile_no % GPM == 0)
                        use_dve = (not use_gp) and DVM and (tile_no % DVM == 0)
                        tile_no += 1
                        if use_dve:
                            t15 = work.tile([P, ROWS], F32, tag="t15")
                            nc.vector.tensor_scalar(
                                t15, s_t, NEG_SLOPE,
                                sj15[:, jt, h:h + 1],
                                op0=ALU.mult, op1=ALU.add,
                            )
                            nc.vector.scalar_tensor_tensor(
                                out=ldst, in0=s_t,
                                scalar=sj_all[:, jt, h:h + 1], in1=t15,
                                op0=ALU.add, op1=ALU.max,
                            )
                        elif use_gp:
                            z_t = work.tile([P, ROWS], F32, tag="z")
                            nc.gpsimd.tensor_scalar(
                                z_t, s_t, sj_all[:, jt, h:h + 1], None,
                                op0=ALU.add,
                            )
                            nc.gpsimd.scalar_tensor_tensor(
                                out=ldst, in0=z_t, scalar=NEG_SLOPE, in1=z_t,
                                op0=ALU.mult, op1=ALU.max,
                            )
                        else:
                            nc.scalar.activation(
                                out=ldst, in_=s_t, func=ACTF.Prelu,
                                bias=sj_all[:, jt, h:h + 1],
                                scale=1.0, alpha=NEG_SLOPE,
                            )
                    eq = quad.tile([P, EG * ROWS], PVDT, tag="eq")
                    nc.scalar.activation(out=eq, in_=lq, func=ACTF.Exp)
                    for q in range(EG):
                        jt = g * EG + q
                        if local_only:
                            nc.vector.tensor_tensor(
                                out=eq[:, q * ROWS:(q + 1) * ROWS],
                                in0=eq[:, q * ROWS:(q + 1) * ROWS],
                                in1=edgeT[jt][:, :], op=ALU.mult,
                            )
                        nc.tensor.matmul(
                            po_h, v_ones[:, jt, h, :],
                            eq[:, q * ROWS:(q + 1) * ROWS],
                            start=(jt == 0), stop=(jt == N_JT - 1),
                        )
                    if g == 0:
                        ensure_bc(h + 1)
                pending_epi.append((h, po_h))
            while pending_epi:
                emit_head_epilogue(*pending_epi.pop())

            # ---- final projection ----------------------------------------
            for ic in range(N_IC):
                psf = ps_a.tile([P, D], F32, tag="pss", name=f"psf_{ic}")
                nc.tensor.matmul(
                    psf, outT[:, ic * P:(ic + 1) * P], wo_sb[:, :],
                    start=True, stop=True,
                )
                fin = work.tile([P, D], F32, tag="fin")
                nc.vector.tensor_copy(out=fin, in_=psf)
                nc.sync.dma_start(out=out_d.ap()[ic * P:(ic + 1) * P, :], in_=fin)

    nc.compile()
    return nc


def _make_in_maps(inputs):
    h = np.ascontiguousarray(np.asarray(inputs["h"], dtype=np.float32))
    sc = np.asarray(inputs["same_cluster"])
    if sc.dtype != np.uint8:
        sc = sc.astype(np.uint8)
    Wa = np.ascontiguousarray(np.asarray(inputs["Wa"], dtype=np.float32))
    Wv = np.ascontiguousarray(np.asarray(inputs["Wv"], dtype=np.float32))
    Wo = np.ascontiguousarray(np.asarray(inputs["Wo"], dtype=np.float32))
    eye = np.eye(P, dtype=np.float32)
    ones = np.ones((1, ROWS), dtype=np.float32)

    in_maps = []
    for c in range(N_CORES):
        b = c // 2
        r0 = (c % 2) * ROWS
        in_maps.append({
            "h_b": h[b],
            "h_rows": np.ascontiguousarray(h[b, r0:r0 + ROWS, :]),
            "sc_rows": np.ascontiguousarray(sc[b, r0:r0 + ROWS, :]),
            "Wa": Wa, "Wv": Wv, "Wo": Wo, "eye": eye, "ones": ones,
        })
    return in_maps


def _build_runner(nc):
    """Persistent jitted shard_map runner (avoids per-call retracing)."""
    import jax
    from jax.sharding import Mesh, PartitionSpec
    from jax.experimental.shard_map import shard_map
    from concourse.bass2jax import (
        _bass_exec_p, install_neuronx_cc_hook, partition_id_tensor,
    )

    install_neuronx_cc_hook()
    partition_name = nc.partition_id_tensor.name if nc.partition_id_tensor else None
    in_names, out_names, out_avals, zero_shapes = [], [], [], []
    for alloc in nc.m.functions[0].allocations:
        if not isinstance(alloc, mybir.MemoryLocationSet):
            continue
        name = alloc.memorylocations[0].name
        if alloc.kind == "ExternalInput":
            if name != partition_name:
                in_names.append(name)
        elif alloc.kind == "ExternalOutput":
            out_names.append(name)
            shape = tuple(alloc.tensor_shape)
            dtype = mybir.dt.np(alloc.dtype)
            out_avals.append(jax.core.ShapedArray(shape, dtype))
            zero_shapes.append((shape, dtype))
    n_params = len(in_names)
    all_in_names = list(in_names) + list(out_names)
    if partition_name is not None:
        all_in_names.append(partition_name)

    def _body(*args):
        operands = list(args)
        if partition_name is not None:
            operands.append(partition_id_tensor())
        outs = _bass_exec_p.bind(
            *operands,
            out_avals=tuple(out_avals),
            in_names=tuple(all_in_names),
            out_names=tuple(out_names),
            lowering_input_output_aliases=(),
            sim_require_finite=True,
            sim_require_nnan=True,
            nc=nc,
        )
        return tuple(outs)

    devices = jax.devices()[:N_CORES]
    mesh = Mesh(np.asarray(devices), ("core",))
    in_specs = (PartitionSpec("core"),) * (n_params + len(out_names))
    out_specs = (PartitionSpec("core"),) * len(out_names)
    fn = jax.jit(
        shard_map(_body, mesh=mesh, in_specs=in_specs, out_specs=out_specs,
                  check_rep=False),
        donate_argnums=tuple(range(n_params, n_params + len(out_names))),
        keep_unused=True,
    )
    return fn, in_names, out_names, zero_shapes


def kernel(h, same_cluster, Wa, Wv, Wo, local_only):
    local_only = int(local_only)
    key = ("prog", local_only)
    if key not in _cache:
        _cache[key] = _build_program(local_only)
    nc = _cache[key]

    in_maps = _make_in_maps({
        "h": h, "same_cluster": same_cluster, "Wa": Wa, "Wv": Wv, "Wo": Wo,
    })

    try:
        rkey = ("runner", local_only)
        if rkey not in _cache:
            _cache[rkey] = _build_runner(nc)
        fn, in_names, out_names, zero_shapes = _cache[rkey]
        concat_in = [
            np.concatenate([np.asarray(in_maps[c][nm]) for c in range(N_CORES)],
                           axis=0)
            for nm in in_names
        ]
        concat_zeros = [
            np.zeros((N_CORES * s[0], *s[1:]), dt) for s, dt in zero_shapes
        ]
        out_arrs = fn(*concat_in, *concat_zeros)
        res_per_core = np.asarray(out_arrs[out_names.index("out_rows")]).reshape(
            N_CORES, ROWS, D
        )
    except Exception:
        res = run_bass_kernel_spmd(nc, in_maps, list(range(N_CORES)))
        res_per_core = np.stack(
            [res.results[c]["out_rows"] for c in range(N_CORES)]
        )

    out = np.empty((B, N, D), dtype=np.float32)
    for c in range(N_CORES):
        b = c // 2
        r0 = (c % 2) * ROWS
        out[b, r0:r0 + ROWS, :] = res_per_core[c]
    return out


if __name__ == "__main__":
    rng = np.random.default_rng(0)
    h = rng.standard_normal((B, N, D), dtype=np.float32)
    sc = rng.integers(0, 2, (B, N, N)).astype(bool)
    Wa = rng.standard_normal((2 * D + 1, H), dtype=np.float32) / np.sqrt(2 * D + 1)
    Wv = rng.standard_normal((D, H * QKV), dtype=np.float32) / np.sqrt(D)
    Wo = rng.standard_normal((H * QKV, D), dtype=np.float32) / np.sqrt(H * QKV)

    out = kernel(h=h, same_cluster=sc, Wa=Wa, Wv=Wv, Wo=Wo, local_only=0)

    Wa_i, Wa_j, w_e = Wa[:D], Wa[D:2 * D], Wa[2 * D]
    s_i = h @ Wa_i
    s_j = h @ Wa_j
    scores = (s_i[:, :, None, :] + s_j[:, None, :, :]
              + sc.astype(np.float32)[..., None] * w_e)
    scores = np.where(scores > 0, scores, NEG_SLOPE * scores)
    scores = np.moveaxis(scores, -1, 1)
    scores = scores - scores.max(axis=-1, keepdims=True)
    e = np.exp(scores)
    alpha = e / e.sum(axis=-1, keepdims=True)
    v = (h @ Wv).reshape(B, N, H, QKV).transpose(0, 2, 1, 3)
    o = np.einsum('bhij,bhjd->bhid', alpha, v)
    o = o.transpose(0, 2, 1, 3).reshape(B, N, H * QKV)
    expected = o @ Wo

    err = np.abs(out - expected)
    rel = np.linalg.norm(out - expected) / np.linalg.norm(expected)
    print(f"rel_err(norm)={rel:.3e} max_abs={err.max():.3e}")
